# revision 9
# baseline (speedup 1.0000x reference)
"""Trainium2 Bass kernel for BlankEmbedding (embedding lookup + blank shift-accumulate).

Reference semantics:
    out = emb[x]                               # [B, S, D]
    preblank[s] = (x[s+1]==BLANK) & (x[s]!=BLANK)   (per row; preblank[S-1]=0)
    out[s] += sum_{k=1..3} preblank[s-k] * emb[x[s-k]]   (zero-pad at row start)

Strategy: data-parallel over the 16384 flattened tokens, 2048 per core.
Each core holds the full table in DRAM and gathers its 2048 rows with
per-partition-index indirect DMAs (16 token tiles of [128, DIM] with
token t = 128*i + p, plus a 3-row halo tile; SWDGE indirect DMA is
limited to 128 indices per instruction).

The shift-accumulate runs on the tensor engine with the base folded in:
out_i = M_i.T @ g_i + (E*w_{i-1}).T @ g_{i-1}, where M_i = I + A*w_i,
A[p,q] = 1 iff 1 <= q-p <= 3 (in-tile shifts), E[p,q] = 1 iff
1 <= q+128-p <= 3 (cross-tile shifts, nonzero only for q<3), and w is
the per-position preblank mask computed on-device from the int32 token
stream in one [128,17] batch. The first tile's cross-term contracts a
[3,3] band against the 3-row halo. All matmul operands are float32r
views of the fp32 data: with a 512-wide moving dim the PE runs at
1 cycle/row (4x the plain-fp32 rate) with near-fp32 precision and no
cast pass. The PSUM results are cast to bf16 alternately on the DVE and
ACT engines (the store issue goes to the opposite sequencer so it never
queues behind its own tile's copy) and stored with HWDGE DMAs at half
the fp32 write traffic; the host upcasts.
"""

import numpy as np

VOCAB = 50257
DIM = 1024
BLANK = 100
B, S = 4, 4096
N_CORES = 8
TOK = B * S                  # 16384 flattened tokens
TPC = TOK // N_CORES         # 2048 tokens per core
P = 128                      # SBUF partitions
NT = TPC // P                # 16 tiles per core
HALO = 3                     # max shift distance
EXT = TPC + HALO + 1         # 2052: 3 halo + 2048 tokens + 1 pad
NMM = DIM // 512             # matmul free-dim chunks per tile
SMALL_HALO = True            # 3-row halo gather (fallback: full 128 rows)

_CACHE = {}


def _shift_consts():
    """lhsT-layout [p_src, q_out] shift bands: A in-tile, E cross-tile, I,
    and E3h for the 3-row halo tile (halo row r = token r-3)."""
    p = np.arange(P)[:, None]
    q = np.arange(P)[None, :]
    a_mat = ((q - p >= 1) & (q - p <= HALO)).astype(np.float32)
    e_mat = ((q + P - p >= 1) & (q + P - p <= HALO)).astype(np.float32)
    i_mat = np.eye(P, dtype=np.float32)
    r = np.arange(P)[:, None]
    q3 = np.arange(HALO)[None, :]
    e3h = ((q3 + HALO - r >= 1) & (q3 + HALO - r <= HALO)).astype(np.float32)
    return a_mat, e_mat[:, :HALO], i_mat, e3h


def _build_nc():
    from concourse import bacc, mybir, tile
    import concourse.bass as bass

    nc = bacc.Bacc(
        "TRN2", target_bir_lowering=False, debug=False, num_devices=N_CORES
    )
    i32 = mybir.dt.int32
    f32 = mybir.dt.float32
    f32r = mybir.dt.float32r
    bf16 = mybir.dt.bfloat16
    NC = NT + 1  # tile columns incl. halo (index 0)

    ix_dram = nc.dram_tensor("ix_pack", [P, 2 * NC], i32, kind="ExternalInput")
    emb = nc.dram_tensor("emb", [VOCAB, DIM], f32, kind="ExternalInput")
    aei_dram = nc.dram_tensor("aei_pack", [P, 2 * P + 2 * HALO], f32,
                              kind="ExternalInput")
    out = nc.dram_tensor("out", [TPC, DIM], bf16, kind="ExternalOutput")

    with tile.TileContext(nc) as tc:
        with (
            tc.tile_pool(name="sbuf", bufs=1) as pool,
            tc.tile_pool(name="psum", bufs=4, space="PSUM") as psum_pool,
        ):
            # ---- packed constant loads on two sequencers in parallel ----
            ixp = pool.tile([P, 2 * NC], i32)
            aei = pool.tile([P, 2 * P + 2 * HALO], f32)
            nc.scalar.dma_start(out=ixp[:], in_=ix_dram[:])
            nc.sync.dma_start(out=aei[:], in_=aei_dram[:])
            ix_all = ixp[:, 0:NC]
            ixn_all = ixp[:, NC: 2 * NC]
            a_sb = aei[:, 0:P]
            i_sb = aei[:, P: 2 * P]
            e_sb = aei[:, 2 * P: 2 * P + HALO]
            eh_sb = aei[:, 2 * P + HALO: 2 * P + 2 * HALO]

            # ---- preblank masks w = isblank(next) & ~isblank(cur) ----
            b_all = pool.tile([P, NC], i32)
            bn_all = pool.tile([P, NC], i32)
            w_all = pool.tile([P, NC], f32)
            nc.vector.tensor_scalar(
                out=b_all[:], in0=ix_all, scalar1=BLANK, scalar2=None,
                op0=mybir.AluOpType.is_equal,
            )
            nc.vector.tensor_scalar(
                out=bn_all[:], in0=ixn_all, scalar1=BLANK, scalar2=None,
                op0=mybir.AluOpType.is_equal,
            )
            nc.vector.tensor_scalar(  # b := 1 - b
                out=b_all[:], in0=b_all[:], scalar1=-1, scalar2=1,
                op0=mybir.AluOpType.mult, op1=mybir.AluOpType.add,
            )
            nc.vector.tensor_tensor(  # bn := bn * (1 - b)
                out=bn_all[:], in0=bn_all[:], in1=b_all[:],
                op=mybir.AluOpType.mult,
            )
            nc.vector.tensor_copy(out=w_all[:], in_=bn_all[:])

            # ---- gathers; halo (tile 0) first so tile 1's cross-term
            # unblocks immediately ----
            emb_r = emb[:].bitcast(f32r)
            if SMALL_HALO:
                g0 = pool.tile([HALO, DIM], f32r, name="g0")
                nc.gpsimd.indirect_dma_start(
                    out=g0[:], out_offset=None, in_=emb_r,
                    in_offset=bass.IndirectOffsetOnAxis(
                        ap=ix_all[0:HALO, 0:1], axis=0
                    ),
                )
            else:
                g0 = pool.tile([P, DIM], f32r, name="g0")
                nc.gpsimd.indirect_dma_start(
                    out=g0[:], out_offset=None, in_=emb_r,
                    in_offset=bass.IndirectOffsetOnAxis(
                        ap=ix_all[:, 0:1], axis=0
                    ),
                )
            g = [g0] + [None] * NT
            for j in range(1, NC):
                g[j] = pool.tile([P, DIM], f32r, name=f"g{j}")
                nc.gpsimd.indirect_dma_start(
                    out=g[j][:], out_offset=None, in_=emb_r,
                    in_offset=bass.IndirectOffsetOnAxis(
                        ap=ix_all[:, j: j + 1], axis=0
                    ),
                )

            # ---- per-tile matmul / cast-copy / store chains ----
            for j in range(1, NC):
                i = j - 1  # output tile index

                m_sb = pool.tile([P, P], f32r, name=f"m{i}")
                nc.vector.scalar_tensor_tensor(  # M = A*w_j + I (fused)
                    out=m_sb[:], in0=a_sb, scalar=w_all[:, j: j + 1],
                    in1=i_sb, op0=mybir.AluOpType.mult,
                    op1=mybir.AluOpType.add,
                )
                if j == 1 and SMALL_HALO:
                    ew_sb = pool.tile([HALO, HALO], f32r, name="ew0")
                    nc.vector.tensor_tensor(  # E3h * w_halo (bcast)
                        out=ew_sb[:], in0=eh_sb[0:HALO, :],
                        in1=w_all[0:HALO, 0:1].to_broadcast([HALO, HALO]),
                        op=mybir.AluOpType.mult,
                    )
                    rhs_prev = g0
                else:
                    ew_sb = pool.tile([P, HALO], f32r, name=f"ew{i}")
                    nc.vector.tensor_tensor(  # Ew = E * w_{j-1} (bcast)
                        out=ew_sb[:], in0=e_sb,
                        in1=w_all[:, j - 1: j].to_broadcast([P, HALO]),
                        op=mybir.AluOpType.mult,
                    )
                    rhs_prev = g[j - 1]

                c = psum_pool.tile([P, DIM], f32, name=f"c{i}", tag="c",
                                   bufs=4)
                for h in range(NMM):
                    sl = slice(512 * h, 512 * (h + 1))
                    nc.tensor.matmul(
                        out=c[:, sl], lhsT=m_sb[:], rhs=g[j][:, sl],
                        start=True, stop=False,
                    )
                    nc.tensor.matmul(
                        out=c[0:HALO, sl], lhsT=ew_sb[:],
                        rhs=rhs_prev[:, sl],
                        start=False, stop=True, skip_group_check=True,
                    )
                o_sb = pool.tile([P, DIM], bf16, name=f"o{i}", tag="o",
                                 bufs=8)
                if i % 2 == 0:
                    nc.scalar.copy(out=o_sb[:], in_=c[:])
                    nc.sync.dma_start(out=out[P * i: P * (i + 1), :],
                                      in_=o_sb[:])
                else:
                    nc.vector.tensor_copy(out=o_sb[:], in_=c[:])
                    nc.scalar.dma_start(out=out[P * i: P * (i + 1), :],
                                        in_=o_sb[:])

    nc.compile()
    return nc


def get_nc():
    if "nc" not in _CACHE:
        _CACHE["nc"] = _build_nc()
    return _CACHE["nc"]


def shard_inputs(x, emb_table):
    """Build per-core in_maps from full inputs."""
    flat = np.ascontiguousarray(np.asarray(x).astype(np.int32).reshape(-1))
    emb_f32 = np.ascontiguousarray(np.asarray(emb_table, dtype=np.float32))
    a_mat, e_mat, i_mat, e3h = _shift_consts()
    aei = np.ascontiguousarray(
        np.concatenate([a_mat, i_mat, e_mat, e3h], axis=1))
    in_maps = []
    for c in range(N_CORES):
        start = c * TPC
        ext = np.zeros(EXT, dtype=np.int32)
        if start % S == 0:
            # row start: blank-filled halo makes the preblank mask 0 there,
            # matching the reference's zero-padded shifts at row boundaries
            ext[:HALO] = BLANK
        else:
            ext[:HALO] = flat[start - HALO: start]
        ext[HALO: HALO + TPC] = flat[start: start + TPC]
        # ext[-1] stays 0: only read to build w at the last position, whose
        # A-matrix row is all-zero (contributions belong to the next core)
        ix_cols = np.zeros((P, NT + 1), dtype=np.int32)
        ixn_cols = np.zeros((P, NT + 1), dtype=np.int32)
        # halo tokens -3..-1 sit at partitions 0..2 of column 0
        ix_cols[0:HALO, 0] = ext[0:HALO]
        ixn_cols[0:HALO, 0] = ext[1: HALO + 1]
        # tile layout: token t = 128*i + p -> column i+1, partition p
        ix_cols[:, 1:] = ext[HALO: HALO + TPC].reshape(NT, P).T
        ixn_cols[:, 1:] = ext[HALO + 1: HALO + 1 + TPC].reshape(NT, P).T
        ix_pack = np.ascontiguousarray(
            np.concatenate([ix_cols, ixn_cols], axis=1))
        in_maps.append(
            {"ix_pack": ix_pack, "emb": emb_f32, "aei_pack": aei}
        )
    return in_maps


def assemble_output(results):
    parts = [np.asarray(results[c]["out"], dtype=np.float32)
             for c in range(N_CORES)]
    return np.concatenate(parts, axis=0).reshape(B, S, DIM)


def kernel(x, emb_table):
    from concourse.bass_utils import run_bass_kernel_spmd

    nc = get_nc()
    in_maps = shard_inputs(x, emb_table)
    res = run_bass_kernel_spmd(nc, in_maps, core_ids=list(range(N_CORES)))
    return assemble_output(res.results)
